# revision 51
# baseline (speedup 1.0000x reference)
"""Trainium2 Bass kernel for AffinityLoss (nn_AffinityLoss_70875550318911).

Math: loss = mean over (n, a, b, l) of BCEWithLogits(aff_map, lb_map) where
aff_map[n,a,b,l] = sum_c lu[n,c,a,l]*lu[n,c,b,l] over 3x3 unfold positions.

Reformulation: pairs (a,b) sharing relative offset d=(di,dj) share one
correlation map D_d[p] = sum_c logits[c,p]*logits[c,p+d]; by symmetry only 13
offsets are needed. Border multiplicities w(p) = sym*wy(di,Y)*wx(dj,X), and
with m = [labels match at offset d] the BCE term reduces to

  total = sum_d sum_p [ w*softplus(D) - (w*m)*D ],   softplus(x)=ln(1+e^x)
  loss  = total / (n * 81 * 382^2)

Layout: channels-on-partitions. Per core, 6 groups (3 y-bands x 2 batches)
of 19 channels -> 114 partitions; free dim = flattened (y, x) band pixels,
so ALL 13 offset shifts are free-dim shifts of ONE resident tile (one input
DMA, no shifted copies). Engine split:
  - DVE (+ Pool for a few offsets) computes the elementwise product tile.
  - TensorE contracts the 19 channels: per 128-column block, a matmul with
    stationary = product block [114,128] and moving = a constant 6-column
    group-selector produces D for 128 pixels x 6 groups straight into PSUM
    (f32, exact sum of bf16 products) - replacing the DVE c-sum add tree.
  - ACT computes softplus as Exp then Ln(bias=1), batched in two phases so
    only 4 activation-table loads occur (exp and ln live in different
    table sets; the merged natural_log_exp set's Ln is broken on HW).
  - DVE scalar_tensor_tensor ops with accum_out form the two weighted sums
    per offset (weights fully host-precomputed per pixel, including all
    border effects - no correction terms needed).
A final ones-matmul reduces the per-partition accumulator strips; each core
returns one unnormalized scalar; host sums and scales.

Emission order is software-pipelined: consumers of offset q's PSUM are
emitted roughly when that PSUM becomes ready, so no engine FIFO head-blocks.
"""
import os
import numpy as np
import ml_dtypes

NCORES = 8
N, C, H, W = 2, 19, 384, 384
KS = 3
BPC = H // NCORES           # 48 owned rows per core
NB = 3                      # y-bands per core
BY = BPC // NB              # 16 owned rows per band
TY = BY + 2                 # 18 band rows incl halo
XW = W + 4                  # padded row width; x' = x + 2
NG = NB * N                 # 6 groups (band, batch)
KP = NG * C                 # 114 partitions
NBLK = 49                   # 128-wide pixel blocks per offset
PF = NBLK * 128             # 6272 product free size
FDC = 2 * XW + 2 + PF + 2   # 7052: max shift 778, +pad (even)
PCOLS = NG * NBLK           # 294 psum columns
NOFF = 13

# (di, dj, sym): di >= 0; for di == 0 only dj >= 0. sym 2 covers (-di,-dj).
OFFSETS = [(0, 0, 1.0), (0, 1, 2.0), (0, 2, 2.0),
           (1, -2, 2.0), (1, -1, 2.0), (1, 0, 2.0), (1, 1, 2.0), (1, 2, 2.0),
           (2, -2, 2.0), (2, -1, 2.0), (2, 0, 2.0), (2, 1, 2.0), (2, 2, 2.0)]

# offsets whose product runs on the GPSIMD (Pool) engine, freeing the DVE
POOL_QS = tuple(int(x) for x in
                os.environ.get("AFF_POOL_QS", "0,1,2").split(",") if x != "")
# after which DVE-product index each pool offset's consumers are emitted
POOL_AT = tuple(int(x) for x in
                os.environ.get("AFF_POOL_AT", "4,7,9").split(",") if x != "")
# trailing columns of the LAST pool offset's product computed by the DVE
# instead (0 disables), and the DVE-product index where that runs
POOL_TAKE = int(os.environ.get("AFF_POOL_TAKE", "1760"))
REM_AT = int(os.environ.get("AFF_REM_AT", "7"))
# leading columns of the LAST DVE offset's product precomputed by the Pool
# after its own products finish (0 disables)
POOL_PRE = int(os.environ.get("AFF_POOL_PRE", "1152"))
# DVE-product index after which the batched phase-A Ln is emitted
LN_SPLIT = int(os.environ.get("AFF_LN_SPLIT", "7"))


def _schedule():
    # -> (pool_qs, pool_at, dve_full, cons_order): emission schedule and
    # the offset consumption order (= wqm/EB/LB column layout order)
    pool_qs = [q for q in POOL_QS if 0 <= q < NOFF]
    dve_full = [q for q in range(NOFF) if q not in pool_qs]
    pool_at = {q: POOL_AT[i] for i, q in enumerate(pool_qs)
               if i < len(POOL_AT)}
    order = []
    for i in range(len(dve_full)):
        if i >= 1:
            order.append(dve_full[i - 1])
        for pq in pool_qs:
            if pool_at.get(pq) == i:
                order.append(pq)
    for pq in pool_qs:
        if pq not in order:
            order.append(pq)
    order.append(dve_full[-1])
    assert sorted(order) == list(range(NOFF)), order
    return pool_qs, pool_at, dve_full, order

BF16 = ml_dtypes.bfloat16

_PROGRAM = None
LAST_RESULTS = None  # BassKernelResults of the most recent run (for profiling)


def _mult_weight(d: int, p: int, size: int = H) -> int:
    """Number of 3x3 window anchors pairing pixel p with p+d along one axis."""
    lo, hi = max(0, -d), 2 - max(d, 0)
    lo2, hi2 = max(lo, p - (size - KS)), min(hi, p)
    return max(0, hi2 - lo2 + 1)


def _build_program():
    import concourse.tile as tile
    from concourse import bacc, mybir
    from concourse.alu_op_type import AluOpType
    from contextlib import ExitStack

    bf = mybir.dt.bfloat16
    f32 = mybir.dt.float32
    A = AluOpType
    AF = mybir.ActivationFunctionType

    nc = bacc.Bacc("TRN2", target_bir_lowering=False, debug=False,
                   num_devices=NCORES)

    lg_d = nc.dram_tensor("lg", [KP, FDC], bf, kind="ExternalInput")
    # consumption-ordered: WQ block [p*PCOLS,...), then all WM blocks
    wqm_d = nc.dram_tensor("wqm", [128, 2 * NOFF * PCOLS], bf,
                           kind="ExternalInput")
    sel_d = nc.dram_tensor("sel", [KP, NG], bf, kind="ExternalInput")
    out = nc.dram_tensor("out", [128, 2 * NOFF], f32, kind="ExternalOutput")
    dbg = None
    if os.environ.get("AFF_DEBUG_ACC"):
        dbg = nc.dram_tensor("dbg", [128, 2 * NOFF], f32,
                             kind="ExternalOutput")

    pool_qs, pool_at, dve_full, cons_order = _schedule()
    wq_pos = {q: i for i, q in enumerate(cons_order)}
    pool_last = pool_qs[-1] if (pool_qs and POOL_TAKE > 0) else -1

    with ExitStack() as ctx:
        tc = ctx.enter_context(tile.TileContext(nc, pool_alloc_mode="stack"))
        singles = ctx.enter_context(tc.tile_pool(name="singles", bufs=1))
        prodp = ctx.enter_context(tc.tile_pool(
            name="prod", bufs=int(os.environ.get("AFF_PROD_BUFS", "3"))))
        poolprodp = ctx.enter_context(tc.tile_pool(
            name="pprod", bufs=min(3, max(1, len(pool_qs)))))
        scr = ctx.enter_context(tc.tile_pool(
            name="scr", bufs=int(os.environ.get("AFF_SCR_BUFS", "2"))))
        psum = ctx.enter_context(tc.tile_pool(
            name="psum", bufs=int(os.environ.get("AFF_PSUM_BUFS", "4")),
            space="PSUM"))

        LG = singles.tile([KP, FDC], bf, name="LG")
        WQM = singles.tile([128, 2 * NOFF * PCOLS], bf, name="WQM")
        SEL = singles.tile([KP, NG], bf, name="SEL")
        acc = singles.tile([128, 2 * NOFF], f32, name="acc")

        nc.vector.memset(acc[:], 0.0)

        # one DMA device: order matters. The LG band gates all compute, so
        # it goes first, laddered in 4 chunks: the first DVE product is
        # emitted in column quarters, each gated only by the chunks it
        # reads. SEL is needed by the first matmul; the first consumed
        # offsets' weights lead, the rest follow.
        S0 = OFFSETS[dve_full[0]][0] * XW + OFFSETS[dve_full[0]][1]
        qtr = PF // 4
        cuts = [0, qtr + S0 + 4, 2 * qtr + S0 + 4, 3 * qtr + S0 + 4, FDC]
        for a, b in zip(cuts[:-1], cuts[1:]):
            nc.sync.dma_start(LG[:, a:b], lg_d[:, a:b])
        nc.sync.dma_start(SEL[:], sel_d[:])
        wq_lead = 2
        nc.sync.dma_start(WQM[:, 0:wq_lead * PCOLS],
                          wqm_d[:, 0:wq_lead * PCOLS])
        nc.sync.dma_start(WQM[:, wq_lead * PCOLS:NOFF * PCOLS],
                          wqm_d[:, wq_lead * PCOLS:NOFF * PCOLS])
        nc.sync.dma_start(WQM[:, NOFF * PCOLS:],
                          wqm_d[:, NOFF * PCOLS:])

        # layout: WQ block cols [p*PCOLS,(p+1)*PCOLS) then WM block
        # NOFF*PCOLS later, p = consumption position (contiguous phases)
        def wq_ap(q):
            p = wq_pos[q]
            return WQM[:, p * PCOLS:(p + 1) * PCOLS]

        def wm_ap(q):
            p = wq_pos[q]
            return WQM[:, (NOFF + p) * PCOLS:(NOFF + p + 1) * PCOLS]

        prods = {}   # q -> product tile (awaiting matmuls)
        psums = {}   # q -> D psum tile (awaiting jd+exp)
        etiles = {}  # q -> exp tile (awaiting ln)
        act_seq = []  # ACT instrs in emission order; chained below so the
        # scheduler can't interleave Ln-table ops into exp-table phases
        # per-offset exp/ln tiles are column slices (consumption order) of
        # two big tiles so phase lns and ws-reductions batch into wide ops
        EB = singles.tile([128, NOFF * PCOLS], bf, name="EB")
        LB = singles.tile([128, NOFF * PCOLS], bf, name="LB")
        DB = singles.tile([128, NOFF * PCOLS], bf, name="DB")

        def esl(q):
            p = wq_pos[q]
            return EB[:, p * PCOLS:(p + 1) * PCOLS]

        def emit_prod(q, engine, pool):
            di, dj, _sym = OFFSETS[q]
            S = di * XW + dj
            prod = pool.tile([KP, PF], bf, tag="prod")
            engine.tensor_tensor(prod[:], LG[:, 0:PF], LG[:, S:S + PF],
                                 A.mult)
            prods[q] = prod

        def emit_mm(q):
            prod = prods.pop(q)
            pt = psum.tile([128, PCOLS], f32, tag="D")
            for k in range(NBLK):
                nc.tensor.matmul(pt[:, NG * k:NG * (k + 1)],
                                 prod[:, 128 * k:128 * (k + 1)],
                                 SEL[:], start=True, stop=True)
            psums[q] = pt

        def emit_jd_exp(q, in_phase_a=False):
            pt = psums.pop(q)
            p = wq_pos[q]
            if q == 0:
                # offset (0,0): D = |v|^2 >= 0 can reach ~80, far past the
                # Ln table's usable range. Labels always self-match (m=1),
                # so w*softplus(D) - w*D = w*ln(1+exp(-D)) exactly: flip the
                # exponent sign (Ln input stays in (1,2]) and the label term
                # cancels - no jd needed (host zeroes its WM block).
                act_seq.append(nc.scalar.activation(esl(q)[:], pt[:],
                                                    AF.Exp, scale=-1.0))
                if in_phase_a:
                    dsl = DB[:, p * PCOLS:(p + 1) * PCOLS]
                    act_seq.append(nc.scalar.activation(dsl[:], pt[:],
                                                        AF.Copy))
            elif in_phase_a:
                # phase-A label terms batch into one wide reduce later; the
                # idle ACT engine parks a bf16 copy of D in DB meanwhile
                act_seq.append(nc.scalar.activation(esl(q)[:], pt[:],
                                                    AF.Exp))
                dsl = DB[:, p * PCOLS:(p + 1) * PCOLS]
                act_seq.append(nc.scalar.activation(dsl[:], pt[:], AF.Copy))
            else:
                # tail offsets: per-offset label term straight from PSUM
                jscr = scr.tile([128, PCOLS], bf, tag="jscr")
                nc.vector.scalar_tensor_tensor(
                    jscr[:], pt[:], 1.0, wm_ap(q), A.bypass, A.mult,
                    accum_out=acc[:, NOFF + p:NOFF + p + 1])
                act_seq.append(nc.scalar.activation(esl(q)[:], pt[:],
                                                    AF.Exp))
            etiles[q] = True

        def _phase_cols(qs):
            ps = sorted(wq_pos[q] for q in qs)
            assert ps == list(range(ps[0], ps[0] + len(ps))), ps
            return ps[0] * PCOLS, (ps[0] + len(ps)) * PCOLS

        def emit_ln_phase(qs):
            # one wide Ln over the phase's contiguous exp columns
            a, b = _phase_cols(qs)
            act_seq.append(nc.scalar.activation(LB[:, a:b], EB[:, a:b],
                                                AF.Ln, bias=1.0))
            for q in qs:
                etiles.pop(q, None)

        def emit_jd_phase(qs):
            # one wide label-term reduce over phase-A's D copies and the
            # contiguous WM blocks (offset 0's WM is zeroed by the host)
            a, b = _phase_cols(qs)
            col = NOFF + a // PCOLS
            jscr = scr.tile([128, NOFF * PCOLS], bf, tag="jscr2")
            nc.vector.scalar_tensor_tensor(
                jscr[:, a:b], DB[:, a:b], 1.0,
                WQM[:, NOFF * PCOLS + a:NOFF * PCOLS + b], A.bypass,
                A.mult, accum_out=acc[:, col:col + 1])

        def emit_ws_phase(qs):
            # one wide weighted-sum reduce over the phase's contiguous Ln
            # and WQ columns; acc column = the phase's first slot
            a, b = _phase_cols(qs)
            col = a // PCOLS
            wscr = scr.tile([128, NOFF * PCOLS], bf, tag="wscr")
            nc.vector.scalar_tensor_tensor(
                wscr[:, a:b], LB[:, a:b], 1.0, WQM[:, a:b], A.bypass,
                A.mult, accum_out=acc[:, col:col + 1])

        # ---- emission schedule ----------------------------------------
        # pool products start immediately and run concurrently with the
        # DVE; the last one leaves its trailing POOL_TAKE columns to a DVE
        # remainder op emitted at loop index REM_AT
        for q in pool_qs:
            di, dj, _sym = OFFSETS[q]
            S = di * XW + dj
            hi = PF - POOL_TAKE if q == pool_last else PF
            prod = poolprodp.tile([KP, PF], bf, tag="prod")
            nc.gpsimd.tensor_tensor(prod[:, 0:hi], LG[:, 0:hi],
                                    LG[:, S:S + hi], A.mult)
            prods[q] = prod
        lastq = dve_full[-1]
        last_prod = None
        if POOL_PRE > 0:
            # after its queue drains, the pool precomputes the head of the
            # last DVE offset's product; the DVE fills in the tail
            di, dj, _sym = OFFSETS[lastq]
            S = di * XW + dj
            last_prod = prodp.tile([KP, PF], bf, tag="prod")
            nc.gpsimd.tensor_tensor(last_prod[:, 0:POOL_PRE],
                                    LG[:, 0:POOL_PRE],
                                    LG[:, S:S + POOL_PRE], A.mult)

        exp_order = []   # offsets in ACT-exp emission order
        phase_a = None   # offsets ln'd in the batched first phase
        jd_a_done = False

        for i, q in enumerate(dve_full):
            if i == 0:
                # first product emitted in column quarters so each runs as
                # soon as the LG DMA chunk covering its reads lands
                di, dj, _sym = OFFSETS[q]
                S = di * XW + dj
                prod = prodp.tile([KP, PF], bf, tag="prod")
                for a in range(0, PF, qtr):
                    nc.vector.tensor_tensor(prod[:, a:a + qtr],
                                            LG[:, a:a + qtr],
                                            LG[:, S + a:S + a + qtr], A.mult)
                prods[q] = prod
            elif q == lastq and last_prod is not None:
                di, dj, _sym = OFFSETS[q]
                S = di * XW + dj
                nc.vector.tensor_tensor(
                    last_prod[:, POOL_PRE:PF], LG[:, POOL_PRE:PF],
                    LG[:, S + POOL_PRE:S + PF], A.mult)
                prods[q] = last_prod
            else:
                emit_prod(q, nc.vector, prodp)
            if i == REM_AT and pool_last >= 0:
                di, dj, _sym = OFFSETS[pool_last]
                S = di * XW + dj
                pl = prods[pool_last]
                nc.vector.tensor_tensor(
                    pl[:, PF - POOL_TAKE:PF], LG[:, PF - POOL_TAKE:PF],
                    LG[:, S + PF - POOL_TAKE:S + PF], A.mult)
            if i >= 1:
                prev = dve_full[i - 1]
                emit_jd_exp(prev, in_phase_a=(phase_a is None))
                exp_order.append(prev)
            for pq in pool_qs:
                if pool_at.get(pq) == i:
                    emit_mm(pq)
                    emit_jd_exp(pq, in_phase_a=(phase_a is None))
                    exp_order.append(pq)
            emit_mm(q)
            if i == LN_SPLIT:
                phase_a = list(exp_order)
                emit_ln_phase(phase_a)
            if i == LN_SPLIT + 1 and phase_a is not None:
                emit_jd_phase(phase_a)
                jd_a_done = True

        # tail: leftover pool offsets, the last DVE offset, then phase B
        # per-offset (short serial chain) while the batched phase-A
        # weighted sum fills the DVE
        for pq in pool_qs:
            if pq in prods:
                emit_mm(pq)
                emit_jd_exp(pq)
                exp_order.append(pq)
        emit_jd_exp(dve_full[-1])
        exp_order.append(dve_full[-1])
        if phase_a is not None and not jd_a_done:
            emit_jd_phase(phase_a)
        phase_b = [qq for qq in exp_order if qq in etiles]
        if os.environ.get("AFF_B_BATCH", "0") == "1":
            if phase_b:
                emit_ln_phase(phase_b)
            if phase_a is not None:
                emit_ws_phase(phase_a)
            if phase_b:
                emit_ws_phase(phase_b)
        else:
            for qq in phase_b:
                emit_ln_phase([qq])
            if phase_a is not None:
                emit_ws_phase(phase_a)
            for qq in phase_b:
                emit_ws_phase([qq])

        from concourse.tile import add_dep_helper
        for i in range(1, len(act_seq)):
            add_dep_helper(act_seq[i].ins, act_seq[i - 1].ins, sync=False,
                           reason="ACT emission order (table-set phases)")

        if dbg is not None:
            nc.sync.dma_start(dbg[:], acc[:])
        # the [128, 26] accumulator goes to the host, which does the final
        # signed sum (cheaper than a matmul+reduce tail on-device)
        nc.sync.dma_start(out[:], acc[:])
    nc.compile()
    return nc


def _host_inputs(logits: np.ndarray, labels: np.ndarray):
    logits = np.asarray(logits, dtype=np.float32)
    labels = np.asarray(labels)
    lg_bf = logits.astype(BF16)

    wy_tab = np.array([[_mult_weight(d, p, H) for p in range(H)]
                       for d in range(3)], dtype=np.float32)
    wx_tab = np.array([[_mult_weight(d, p, W) for p in range(W)]
                       for d in range(-2, 3)], dtype=np.float32)

    sel = np.zeros((KP, NG), dtype=BF16)
    for g in range(NG):
        sel[g * C:(g + 1) * C, g] = 1.0

    # pixel coordinate tables for the block layout
    f_idx = np.arange(PF)
    y_f = f_idx // XW              # band-local y
    xp_f = f_idx % XW              # padded x'
    part_f = f_idx % 128
    blk_f = f_idx // 128
    x_f = xp_f - 2
    valid_f = (y_f < BY) & (x_f >= 0) & (x_f < W)

    in_maps = []
    for core in range(NCORES):
        r0 = core * BPC
        lg = np.zeros((KP, FDC), dtype=BF16)
        band = np.zeros((NB, N, C, TY, XW), dtype=BF16)
        for b in range(NB):
            rows = max(0, min(TY, H - (r0 + BY * b)))
            band[b, :, :, :rows, 2:2 + W] = \
                lg_bf[:, :, r0 + BY * b:r0 + BY * b + rows, :]
        lg[:, 0:TY * XW] = band.reshape(KP, TY * XW)
        in_map = {"lg": lg, "sel": sel}

        cons_order = _schedule()[-1]
        wq_pos = {q: i for i, q in enumerate(cons_order)}
        wqm = np.zeros((128, 2 * NOFF * PCOLS), dtype=np.float32)
        for q, (di, dj, sym) in enumerate(OFFSETS):
            for g in range(NG):
                b, n = g // N, g % N
                Y = r0 + BY * b + y_f
                X = x_f
                Yc = np.clip(Y, 0, H - 1)
                Xc = np.clip(X, 0, W - 1)
                Y2 = np.clip(Y + di, 0, H - 1)
                X2 = np.clip(X + dj, 0, W - 1)
                w = sym * wy_tab[di, Yc] * wx_tab[dj + 2, Xc] * valid_f
                m = (labels[n, Yc, Xc] == labels[n, Y2, X2])
                cols = wq_pos[q] * PCOLS + NG * blk_f + g
                wqm[part_f, cols] = w
                # offset (0,0)'s label term cancels into its softplus form;
                # its WM block must be zero for the batched phase-A reduce
                wqm[part_f, cols + NOFF * PCOLS] = 0.0 if q == 0 else w * m
        in_map["wqm"] = wqm.astype(BF16)
        in_maps.append(in_map)
    return in_maps


def kernel(logits: np.ndarray, labels: np.ndarray) -> np.ndarray:
    global _PROGRAM, LAST_RESULTS
    from concourse.bass_utils import run_bass_kernel_spmd

    if _PROGRAM is None:
        _PROGRAM = _build_program()

    in_maps = _host_inputs(logits, labels)
    trace = bool(int(os.environ.get("AFF_TRACE", "0")))
    results = run_bass_kernel_spmd(
        _PROGRAM, in_maps, core_ids=list(range(NCORES)), trace=trace)
    LAST_RESULTS = results

    total = 0.0
    for r in results.results:
        a = np.asarray(r["out"], dtype=np.float64)
        total += float(a[:, 0:NOFF].sum() - a[:, NOFF:2 * NOFF].sum())
    Lwin = (H - KS + 1) * (W - KS + 1)
    return np.float32(total / (N * KS**4 * Lwin))


# revision 54
# speedup vs baseline: 1.0051x; 1.0051x over previous
"""Trainium2 Bass kernel for AffinityLoss (nn_AffinityLoss_70875550318911).

Math: loss = mean over (n, a, b, l) of BCEWithLogits(aff_map, lb_map) where
aff_map[n,a,b,l] = sum_c lu[n,c,a,l]*lu[n,c,b,l] over 3x3 unfold positions.

Reformulation: pairs (a,b) sharing relative offset d=(di,dj) share one
correlation map D_d[p] = sum_c logits[c,p]*logits[c,p+d]; by symmetry only 13
offsets are needed. Border multiplicities w(p) = sym*wy(di,Y)*wx(dj,X), and
with m = [labels match at offset d] the BCE term reduces to

  total = sum_d sum_p [ w*softplus(D) - (w*m)*D ],   softplus(x)=ln(1+e^x)
  loss  = total / (n * 81 * 382^2)

Layout: channels-on-partitions. Per core, 6 groups (3 y-bands x 2 batches)
of 19 channels -> 114 partitions; free dim = flattened (y, x) band pixels,
so ALL 13 offset shifts are free-dim shifts of ONE resident tile (one input
DMA, no shifted copies). Engine split:
  - DVE (+ Pool for a few offsets) computes the elementwise product tile.
  - TensorE contracts the 19 channels: per 128-column block, a matmul with
    stationary = product block [114,128] and moving = a constant 6-column
    group-selector produces D for 128 pixels x 6 groups straight into PSUM
    (f32, exact sum of bf16 products) - replacing the DVE c-sum add tree.
  - ACT computes softplus as Exp then Ln(bias=1), batched in two phases so
    only 4 activation-table loads occur (exp and ln live in different
    table sets; the merged natural_log_exp set's Ln is broken on HW).
  - DVE scalar_tensor_tensor ops with accum_out form the two weighted sums
    per offset (weights fully host-precomputed per pixel, including all
    border effects - no correction terms needed).
A final ones-matmul reduces the per-partition accumulator strips; each core
returns one unnormalized scalar; host sums and scales.

Emission order is software-pipelined: consumers of offset q's PSUM are
emitted roughly when that PSUM becomes ready, so no engine FIFO head-blocks.
"""
import os
import numpy as np
import ml_dtypes

NCORES = 8
N, C, H, W = 2, 19, 384, 384
KS = 3
BPC = H // NCORES           # 48 owned rows per core
NB = 3                      # y-bands per core
BY = BPC // NB              # 16 owned rows per band
TY = BY + 2                 # 18 band rows incl halo
XW = W + 4                  # padded row width; x' = x + 2
NG = NB * N                 # 6 groups (band, batch)
KP = NG * C                 # 114 partitions
NBLK = 49                   # 128-wide pixel blocks per offset
PF = NBLK * 128             # 6272 product free size
FDC = 2 * XW + 2 + PF + 2   # 7052: max shift 778, +pad (even)
PCOLS = NG * NBLK           # 294 psum columns
NOFF = 13

# (di, dj, sym): di >= 0; for di == 0 only dj >= 0. sym 2 covers (-di,-dj).
OFFSETS = [(0, 0, 1.0), (0, 1, 2.0), (0, 2, 2.0),
           (1, -2, 2.0), (1, -1, 2.0), (1, 0, 2.0), (1, 1, 2.0), (1, 2, 2.0),
           (2, -2, 2.0), (2, -1, 2.0), (2, 0, 2.0), (2, 1, 2.0), (2, 2, 2.0)]

# offsets whose product runs on the GPSIMD (Pool) engine, freeing the DVE
POOL_QS = tuple(int(x) for x in
                os.environ.get("AFF_POOL_QS", "0,1,2").split(",") if x != "")
# after which DVE-product index each pool offset's consumers are emitted
POOL_AT = tuple(int(x) for x in
                os.environ.get("AFF_POOL_AT", "4,7,9").split(",") if x != "")
# trailing columns of the LAST pool offset's product computed by the DVE
# instead (0 disables), and the DVE-product index where that runs
POOL_TAKE = int(os.environ.get("AFF_POOL_TAKE", "2432"))
REM_AT = int(os.environ.get("AFF_REM_AT", "7"))
# leading columns of the LAST DVE offset's product precomputed by the Pool
# after its own products finish (0 disables)
POOL_PRE = int(os.environ.get("AFF_POOL_PRE", "1152"))
# DVE-product index after which the batched phase-A Ln is emitted
LN_SPLIT = int(os.environ.get("AFF_LN_SPLIT", "7"))


def _schedule():
    # -> (pool_qs, pool_at, dve_full, cons_order): emission schedule and
    # the offset consumption order (= wqm/EB/LB column layout order)
    pool_qs = [q for q in POOL_QS if 0 <= q < NOFF]
    dve_full = [q for q in range(NOFF) if q not in pool_qs]
    pool_at = {q: POOL_AT[i] for i, q in enumerate(pool_qs)
               if i < len(POOL_AT)}
    order = []
    for i in range(len(dve_full)):
        if i >= 1:
            order.append(dve_full[i - 1])
        for pq in pool_qs:
            if pool_at.get(pq) == i:
                order.append(pq)
    for pq in pool_qs:
        if pq not in order:
            order.append(pq)
    order.append(dve_full[-1])
    assert sorted(order) == list(range(NOFF)), order
    return pool_qs, pool_at, dve_full, order

BF16 = ml_dtypes.bfloat16

_PROGRAM = None
LAST_RESULTS = None  # BassKernelResults of the most recent run (for profiling)


def _mult_weight(d: int, p: int, size: int = H) -> int:
    """Number of 3x3 window anchors pairing pixel p with p+d along one axis."""
    lo, hi = max(0, -d), 2 - max(d, 0)
    lo2, hi2 = max(lo, p - (size - KS)), min(hi, p)
    return max(0, hi2 - lo2 + 1)


def _build_program():
    import concourse.tile as tile
    from concourse import bacc, mybir
    from concourse.alu_op_type import AluOpType
    from contextlib import ExitStack

    bf = mybir.dt.bfloat16
    f32 = mybir.dt.float32
    A = AluOpType
    AF = mybir.ActivationFunctionType

    nc = bacc.Bacc("TRN2", target_bir_lowering=False, debug=False,
                   num_devices=NCORES)

    lg_d = nc.dram_tensor("lg", [KP, FDC], bf, kind="ExternalInput")
    # consumption-ordered: WQ block [p*PCOLS,...), then all WM blocks
    wqm_d = nc.dram_tensor("wqm", [128, 2 * NOFF * PCOLS], bf,
                           kind="ExternalInput")
    sel_d = nc.dram_tensor("sel", [KP, NG], bf, kind="ExternalInput")
    out = nc.dram_tensor("out", [128, 2 * NOFF], f32, kind="ExternalOutput")
    out2 = nc.dram_tensor("out2", [128, 16], f32, kind="ExternalOutput")
    dbg = None
    if os.environ.get("AFF_DEBUG_ACC"):
        dbg = nc.dram_tensor("dbg", [128, 2 * NOFF], f32,
                             kind="ExternalOutput")

    pool_qs, pool_at, dve_full, cons_order = _schedule()
    wq_pos = {q: i for i, q in enumerate(cons_order)}
    pool_last = pool_qs[-1] if (pool_qs and POOL_TAKE > 0) else -1

    with ExitStack() as ctx:
        tc = ctx.enter_context(tile.TileContext(nc, pool_alloc_mode="stack"))
        singles = ctx.enter_context(tc.tile_pool(name="singles", bufs=1))
        prodp = ctx.enter_context(tc.tile_pool(
            name="prod", bufs=int(os.environ.get("AFF_PROD_BUFS", "3"))))
        poolprodp = ctx.enter_context(tc.tile_pool(
            name="pprod", bufs=min(3, max(1, len(pool_qs)))))
        scr = ctx.enter_context(tc.tile_pool(
            name="scr", bufs=int(os.environ.get("AFF_SCR_BUFS", "2"))))
        psum = ctx.enter_context(tc.tile_pool(
            name="psum", bufs=int(os.environ.get("AFF_PSUM_BUFS", "4")),
            space="PSUM"))
        psumf = ctx.enter_context(tc.tile_pool(
            name="psumf", bufs=1, space="PSUM"))

        LG = singles.tile([KP, FDC], bf, name="LG")
        ones128 = singles.tile([128, 1], bf, name="ones128")
        fp = psumf.tile([128, 16], f32, name="fp")
        fcp = singles.tile([128, 16], f32, name="fcp")
        WQM = singles.tile([128, 2 * NOFF * PCOLS], bf, name="WQM")
        SEL = singles.tile([KP, NG], bf, name="SEL")
        acc = singles.tile([128, 2 * NOFF], f32, name="acc")

        nc.vector.memset(acc[:], 0.0)
        nc.vector.memset(ones128[:], 1.0)

        # one DMA device: order matters. The LG band gates all compute, so
        # it goes first, laddered in 4 chunks: the first DVE product is
        # emitted in column quarters, each gated only by the chunks it
        # reads. SEL is needed by the first matmul; the first consumed
        # offsets' weights lead, the rest follow.
        S0 = OFFSETS[dve_full[0]][0] * XW + OFFSETS[dve_full[0]][1]
        qtr = PF // 4
        cuts = [0, qtr + S0 + 4, 2 * qtr + S0 + 4, 3 * qtr + S0 + 4, FDC]
        for a, b in zip(cuts[:-1], cuts[1:]):
            nc.sync.dma_start(LG[:, a:b], lg_d[:, a:b])
        nc.sync.dma_start(SEL[:], sel_d[:])
        wq_lead = 2
        nc.sync.dma_start(WQM[:, 0:wq_lead * PCOLS],
                          wqm_d[:, 0:wq_lead * PCOLS])
        nc.sync.dma_start(WQM[:, wq_lead * PCOLS:NOFF * PCOLS],
                          wqm_d[:, wq_lead * PCOLS:NOFF * PCOLS])
        nc.sync.dma_start(WQM[:, NOFF * PCOLS:],
                          wqm_d[:, NOFF * PCOLS:])

        # layout: WQ block cols [p*PCOLS,(p+1)*PCOLS) then WM block
        # NOFF*PCOLS later, p = consumption position (contiguous phases)
        def wq_ap(q):
            p = wq_pos[q]
            return WQM[:, p * PCOLS:(p + 1) * PCOLS]

        def wm_ap(q):
            p = wq_pos[q]
            return WQM[:, (NOFF + p) * PCOLS:(NOFF + p + 1) * PCOLS]

        prods = {}   # q -> product tile (awaiting matmuls)
        psums = {}   # q -> D psum tile (awaiting jd+exp)
        etiles = {}  # q -> exp tile (awaiting ln)
        act_seq = []  # ACT instrs in emission order; chained below so the
        # scheduler can't interleave Ln-table ops into exp-table phases
        # per-offset exp/ln tiles are column slices (consumption order) of
        # two big tiles so phase lns and ws-reductions batch into wide ops
        EB = singles.tile([128, NOFF * PCOLS], bf, name="EB")
        LB = singles.tile([128, NOFF * PCOLS], bf, name="LB")
        DB = singles.tile([128, NOFF * PCOLS], bf, name="DB")

        def esl(q):
            p = wq_pos[q]
            return EB[:, p * PCOLS:(p + 1) * PCOLS]

        def emit_prod(q, engine, pool):
            di, dj, _sym = OFFSETS[q]
            S = di * XW + dj
            prod = pool.tile([KP, PF], bf, tag="prod")
            engine.tensor_tensor(prod[:], LG[:, 0:PF], LG[:, S:S + PF],
                                 A.mult)
            prods[q] = prod

        def emit_mm(q):
            prod = prods.pop(q)
            pt = psum.tile([128, PCOLS], f32, tag="D")
            for k in range(NBLK):
                nc.tensor.matmul(pt[:, NG * k:NG * (k + 1)],
                                 prod[:, 128 * k:128 * (k + 1)],
                                 SEL[:], start=True, stop=True)
            psums[q] = pt

        def emit_jd_exp(q, in_phase_a=False):
            pt = psums.pop(q)
            p = wq_pos[q]
            if q == 0:
                # offset (0,0): D = |v|^2 >= 0 can reach ~80, far past the
                # Ln table's usable range. Labels always self-match (m=1),
                # so w*softplus(D) - w*D = w*ln(1+exp(-D)) exactly: flip the
                # exponent sign (Ln input stays in (1,2]) and the label term
                # cancels - no jd needed (host zeroes its WM block).
                act_seq.append(nc.scalar.activation(esl(q)[:], pt[:],
                                                    AF.Exp, scale=-1.0))
                if in_phase_a:
                    dsl = DB[:, p * PCOLS:(p + 1) * PCOLS]
                    act_seq.append(nc.scalar.activation(dsl[:], pt[:],
                                                        AF.Copy))
            elif in_phase_a:
                # phase-A label terms batch into one wide reduce later; the
                # idle ACT engine parks a bf16 copy of D in DB meanwhile
                act_seq.append(nc.scalar.activation(esl(q)[:], pt[:],
                                                    AF.Exp))
                dsl = DB[:, p * PCOLS:(p + 1) * PCOLS]
                act_seq.append(nc.scalar.activation(dsl[:], pt[:], AF.Copy))
            else:
                # tail offsets: per-offset label term straight from PSUM
                jscr = scr.tile([128, PCOLS], bf, tag="jscr")
                nc.vector.scalar_tensor_tensor(
                    jscr[:], pt[:], 1.0, wm_ap(q), A.bypass, A.mult,
                    accum_out=acc[:, NOFF + p:NOFF + p + 1])
                act_seq.append(nc.scalar.activation(esl(q)[:], pt[:],
                                                    AF.Exp))
            etiles[q] = True

        def _phase_cols(qs):
            ps = sorted(wq_pos[q] for q in qs)
            assert ps == list(range(ps[0], ps[0] + len(ps))), ps
            return ps[0] * PCOLS, (ps[0] + len(ps)) * PCOLS

        def emit_ln_phase(qs):
            # one wide Ln over the phase's contiguous exp columns
            a, b = _phase_cols(qs)
            act_seq.append(nc.scalar.activation(LB[:, a:b], EB[:, a:b],
                                                AF.Ln, bias=1.0))
            for q in qs:
                etiles.pop(q, None)

        def _pe_colsum(src_ap, a, b, fcol):
            # per-partition column sum via TensorE ones-contraction,
            # PSUM-accumulated over 128-column chunks
            n = b - a
            chunks = [(a + j * 128, min(a + (j + 1) * 128, b))
                      for j in range((n + 127) // 128)]
            for ci, (c0, c1) in enumerate(chunks):
                nc.tensor.matmul(fp[0:c1 - c0, fcol:fcol + 1],
                                 src_ap[:, c0:c1], ones128[:],
                                 start=(ci == 0), stop=(ci == len(chunks) - 1))

        def emit_jd_phase(qs):
            # phase-A label terms: 2x-mode multiply of the D copies by the
            # contiguous WM blocks (offset 0's WM zeroed host-side), with
            # the column sum on the idle TensorE
            a, b = _phase_cols(qs)
            jscr = scr.tile([128, NOFF * PCOLS], bf, tag="jscr2")
            nc.vector.tensor_tensor(
                jscr[:, a:b], DB[:, a:b],
                WQM[:, NOFF * PCOLS + a:NOFF * PCOLS + b], A.mult)
            _pe_colsum(jscr, a, b, 0)

        def emit_ws_phase(qs, wide=False):
            a, b = _phase_cols(qs)
            col = a // PCOLS
            wscr = scr.tile([128, NOFF * PCOLS], bf, tag="wscr")
            if wide:
                nc.vector.tensor_tensor(
                    wscr[:, a:b], LB[:, a:b], WQM[:, a:b], A.mult)
                _pe_colsum(wscr, a, b, 8)
            else:
                nc.vector.scalar_tensor_tensor(
                    wscr[:, a:b], LB[:, a:b], 1.0, WQM[:, a:b], A.bypass,
                    A.mult, accum_out=acc[:, col:col + 1])

        # ---- emission schedule ----------------------------------------
        # pool products start immediately and run concurrently with the
        # DVE; the last one leaves its trailing POOL_TAKE columns to a DVE
        # remainder op emitted at loop index REM_AT
        for q in pool_qs:
            di, dj, _sym = OFFSETS[q]
            S = di * XW + dj
            hi = PF - POOL_TAKE if q == pool_last else PF
            prod = poolprodp.tile([KP, PF], bf, tag="prod")
            nc.gpsimd.tensor_tensor(prod[:, 0:hi], LG[:, 0:hi],
                                    LG[:, S:S + hi], A.mult)
            prods[q] = prod
        lastq = dve_full[-1]
        last_prod = None
        if POOL_PRE > 0:
            # after its queue drains, the pool precomputes the head of the
            # last DVE offset's product; the DVE fills in the tail
            di, dj, _sym = OFFSETS[lastq]
            S = di * XW + dj
            last_prod = prodp.tile([KP, PF], bf, tag="prod")
            nc.gpsimd.tensor_tensor(last_prod[:, 0:POOL_PRE],
                                    LG[:, 0:POOL_PRE],
                                    LG[:, S:S + POOL_PRE], A.mult)

        exp_order = []   # offsets in ACT-exp emission order
        phase_a = None   # offsets ln'd in the batched first phase
        jd_a_done = False

        for i, q in enumerate(dve_full):
            if i == 0:
                # first product emitted in column quarters so each runs as
                # soon as the LG DMA chunk covering its reads lands
                di, dj, _sym = OFFSETS[q]
                S = di * XW + dj
                prod = prodp.tile([KP, PF], bf, tag="prod")
                for a in range(0, PF, qtr):
                    nc.vector.tensor_tensor(prod[:, a:a + qtr],
                                            LG[:, a:a + qtr],
                                            LG[:, S + a:S + a + qtr], A.mult)
                prods[q] = prod
            elif q == lastq and last_prod is not None:
                di, dj, _sym = OFFSETS[q]
                S = di * XW + dj
                nc.vector.tensor_tensor(
                    last_prod[:, POOL_PRE:PF], LG[:, POOL_PRE:PF],
                    LG[:, S + POOL_PRE:S + PF], A.mult)
                prods[q] = last_prod
            else:
                emit_prod(q, nc.vector, prodp)
            if i == REM_AT and pool_last >= 0:
                di, dj, _sym = OFFSETS[pool_last]
                S = di * XW + dj
                pl = prods[pool_last]
                nc.vector.tensor_tensor(
                    pl[:, PF - POOL_TAKE:PF], LG[:, PF - POOL_TAKE:PF],
                    LG[:, S + PF - POOL_TAKE:S + PF], A.mult)
            if i >= 1:
                prev = dve_full[i - 1]
                emit_jd_exp(prev, in_phase_a=(phase_a is None))
                exp_order.append(prev)
            for pq in pool_qs:
                if pool_at.get(pq) == i:
                    emit_mm(pq)
                    emit_jd_exp(pq, in_phase_a=(phase_a is None))
                    exp_order.append(pq)
            emit_mm(q)
            if i == LN_SPLIT:
                phase_a = list(exp_order)
                emit_ln_phase(phase_a)
            if i == LN_SPLIT + 1 and phase_a is not None:
                emit_jd_phase(phase_a)
                jd_a_done = True

        # tail: leftover pool offsets, the last DVE offset, then phase B
        # per-offset (short serial chain) while the batched phase-A
        # weighted sum fills the DVE
        for pq in pool_qs:
            if pq in prods:
                emit_mm(pq)
                emit_jd_exp(pq)
                exp_order.append(pq)
        emit_jd_exp(dve_full[-1])
        exp_order.append(dve_full[-1])
        if phase_a is not None and not jd_a_done:
            emit_jd_phase(phase_a)
        phase_b = [qq for qq in exp_order if qq in etiles]
        if os.environ.get("AFF_B_BATCH", "0") == "1":
            if phase_b:
                emit_ln_phase(phase_b)
            if phase_a is not None:
                emit_ws_phase(phase_a, wide=True)
            if phase_b:
                emit_ws_phase(phase_b)
        else:
            for qq in phase_b:
                emit_ln_phase([qq])
            if phase_a is not None:
                emit_ws_phase(phase_a, wide=True)
            for qq in phase_b:
                emit_ws_phase([qq])

        from concourse.tile import add_dep_helper
        for i in range(1, len(act_seq)):
            add_dep_helper(act_seq[i].ins, act_seq[i - 1].ins, sync=False,
                           reason="ACT emission order (table-set phases)")

        if dbg is not None:
            nc.sync.dma_start(dbg[:], acc[:])
        # the [128, 26] accumulator goes to the host, which does the final
        # signed sum (cheaper than a matmul+reduce tail on-device)
        nc.vector.tensor_copy(fcp[:], fp[:])
        nc.sync.dma_start(out[:], acc[:])
        nc.scalar.dma_start(out2[:], fcp[:])
    nc.compile()
    return nc


def _host_inputs(logits: np.ndarray, labels: np.ndarray):
    logits = np.asarray(logits, dtype=np.float32)
    labels = np.asarray(labels)
    lg_bf = logits.astype(BF16)

    wy_tab = np.array([[_mult_weight(d, p, H) for p in range(H)]
                       for d in range(3)], dtype=np.float32)
    wx_tab = np.array([[_mult_weight(d, p, W) for p in range(W)]
                       for d in range(-2, 3)], dtype=np.float32)

    sel = np.zeros((KP, NG), dtype=BF16)
    for g in range(NG):
        sel[g * C:(g + 1) * C, g] = 1.0

    # pixel coordinate tables for the block layout
    f_idx = np.arange(PF)
    y_f = f_idx // XW              # band-local y
    xp_f = f_idx % XW              # padded x'
    part_f = f_idx % 128
    blk_f = f_idx // 128
    x_f = xp_f - 2
    valid_f = (y_f < BY) & (x_f >= 0) & (x_f < W)

    in_maps = []
    for core in range(NCORES):
        r0 = core * BPC
        lg = np.zeros((KP, FDC), dtype=BF16)
        band = np.zeros((NB, N, C, TY, XW), dtype=BF16)
        for b in range(NB):
            rows = max(0, min(TY, H - (r0 + BY * b)))
            band[b, :, :, :rows, 2:2 + W] = \
                lg_bf[:, :, r0 + BY * b:r0 + BY * b + rows, :]
        lg[:, 0:TY * XW] = band.reshape(KP, TY * XW)
        in_map = {"lg": lg, "sel": sel}

        cons_order = _schedule()[-1]
        wq_pos = {q: i for i, q in enumerate(cons_order)}
        wqm = np.zeros((128, 2 * NOFF * PCOLS), dtype=np.float32)
        for q, (di, dj, sym) in enumerate(OFFSETS):
            for g in range(NG):
                b, n = g // N, g % N
                Y = r0 + BY * b + y_f
                X = x_f
                Yc = np.clip(Y, 0, H - 1)
                Xc = np.clip(X, 0, W - 1)
                Y2 = np.clip(Y + di, 0, H - 1)
                X2 = np.clip(X + dj, 0, W - 1)
                w = sym * wy_tab[di, Yc] * wx_tab[dj + 2, Xc] * valid_f
                m = (labels[n, Yc, Xc] == labels[n, Y2, X2])
                cols = wq_pos[q] * PCOLS + NG * blk_f + g
                wqm[part_f, cols] = w
                # offset (0,0)'s label term cancels into its softplus form;
                # its WM block must be zero for the batched phase-A reduce
                wqm[part_f, cols + NOFF * PCOLS] = 0.0 if q == 0 else w * m
        in_map["wqm"] = wqm.astype(BF16)
        in_maps.append(in_map)
    return in_maps


def kernel(logits: np.ndarray, labels: np.ndarray) -> np.ndarray:
    global _PROGRAM, LAST_RESULTS
    from concourse.bass_utils import run_bass_kernel_spmd

    if _PROGRAM is None:
        _PROGRAM = _build_program()

    in_maps = _host_inputs(logits, labels)
    trace = bool(int(os.environ.get("AFF_TRACE", "0")))
    results = run_bass_kernel_spmd(
        _PROGRAM, in_maps, core_ids=list(range(NCORES)), trace=trace)
    LAST_RESULTS = results

    total = 0.0
    for r in results.results:
        a = np.asarray(r["out"], dtype=np.float64)
        b = np.asarray(r["out2"], dtype=np.float64)
        total += float(a[:, 0:NOFF].sum() - a[:, NOFF:2 * NOFF].sum())
        total += float(b[:, 8].sum() - b[:, 0].sum())
    Lwin = (H - KS + 1) * (W - KS + 1)
    return np.float32(total / (N * KS**4 * Lwin))


# revision 55
# speedup vs baseline: 1.0099x; 1.0047x over previous
"""Trainium2 Bass kernel for AffinityLoss (nn_AffinityLoss_70875550318911).

Math: loss = mean over (n, a, b, l) of BCEWithLogits(aff_map, lb_map) where
aff_map[n,a,b,l] = sum_c lu[n,c,a,l]*lu[n,c,b,l] over 3x3 unfold positions.

Reformulation: pairs (a,b) sharing relative offset d=(di,dj) share one
correlation map D_d[p] = sum_c logits[c,p]*logits[c,p+d]; by symmetry only 13
offsets are needed. Border multiplicities w(p) = sym*wy(di,Y)*wx(dj,X), and
with m = [labels match at offset d] the BCE term reduces to

  total = sum_d sum_p [ w*softplus(D) - (w*m)*D ],   softplus(x)=ln(1+e^x)
  loss  = total / (n * 81 * 382^2)

Layout: channels-on-partitions. Per core, 6 groups (3 y-bands x 2 batches)
of 19 channels -> 114 partitions; free dim = flattened (y, x) band pixels,
so ALL 13 offset shifts are free-dim shifts of ONE resident tile (one input
DMA, no shifted copies). Engine split:
  - DVE (+ Pool for a few offsets) computes the elementwise product tile.
  - TensorE contracts the 19 channels: per 128-column block, a matmul with
    stationary = product block [114,128] and moving = a constant 6-column
    group-selector produces D for 128 pixels x 6 groups straight into PSUM
    (f32, exact sum of bf16 products) - replacing the DVE c-sum add tree.
  - ACT computes softplus as Exp then Ln(bias=1), batched in two phases so
    only 4 activation-table loads occur (exp and ln live in different
    table sets; the merged natural_log_exp set's Ln is broken on HW).
  - DVE scalar_tensor_tensor ops with accum_out form the two weighted sums
    per offset (weights fully host-precomputed per pixel, including all
    border effects - no correction terms needed).
A final ones-matmul reduces the per-partition accumulator strips; each core
returns one unnormalized scalar; host sums and scales.

Emission order is software-pipelined: consumers of offset q's PSUM are
emitted roughly when that PSUM becomes ready, so no engine FIFO head-blocks.
"""
import os
import numpy as np
import ml_dtypes

NCORES = 8
N, C, H, W = 2, 19, 384, 384
KS = 3
BPC = H // NCORES           # 48 owned rows per core
NB = 3                      # y-bands per core
BY = BPC // NB              # 16 owned rows per band
TY = BY + 2                 # 18 band rows incl halo
XW = W + 4                  # padded row width; x' = x + 2
NG = NB * N                 # 6 groups (band, batch)
KP = NG * C                 # 114 partitions
NBLK = 49                   # 128-wide pixel blocks per offset
PF = NBLK * 128             # 6272 product free size
FDC = 2 * XW + 2 + PF + 2   # 7052: max shift 778, +pad (even)
PCOLS = NG * NBLK           # 294 psum columns
NOFF = 13

# (di, dj, sym): di >= 0; for di == 0 only dj >= 0. sym 2 covers (-di,-dj).
OFFSETS = [(0, 0, 1.0), (0, 1, 2.0), (0, 2, 2.0),
           (1, -2, 2.0), (1, -1, 2.0), (1, 0, 2.0), (1, 1, 2.0), (1, 2, 2.0),
           (2, -2, 2.0), (2, -1, 2.0), (2, 0, 2.0), (2, 1, 2.0), (2, 2, 2.0)]

# offsets whose product runs on the GPSIMD (Pool) engine, freeing the DVE
POOL_QS = tuple(int(x) for x in
                os.environ.get("AFF_POOL_QS", "0,1,2").split(",") if x != "")
# after which DVE-product index each pool offset's consumers are emitted
POOL_AT = tuple(int(x) for x in
                os.environ.get("AFF_POOL_AT", "4,7,9").split(",") if x != "")
# trailing columns of the LAST pool offset's product computed by the DVE
# instead (0 disables), and the DVE-product index where that runs
POOL_TAKE = int(os.environ.get("AFF_POOL_TAKE", "2432"))
REM_AT = int(os.environ.get("AFF_REM_AT", "7"))
# leading columns of the LAST DVE offset's product precomputed by the Pool
# after its own products finish (0 disables)
POOL_PRE = int(os.environ.get("AFF_POOL_PRE", "1152"))
# DVE-product index after which the batched phase-A Ln is emitted
LN_SPLIT = int(os.environ.get("AFF_LN_SPLIT", "7"))


def _schedule():
    # -> (pool_qs, pool_at, dve_full, cons_order): emission schedule and
    # the offset consumption order (= wqm/EB/LB column layout order)
    pool_qs = [q for q in POOL_QS if 0 <= q < NOFF]
    dve_full = [q for q in range(NOFF) if q not in pool_qs]
    pool_at = {q: POOL_AT[i] for i, q in enumerate(pool_qs)
               if i < len(POOL_AT)}
    order = []
    for i in range(len(dve_full)):
        if i >= 1:
            order.append(dve_full[i - 1])
        for pq in pool_qs:
            if pool_at.get(pq) == i:
                order.append(pq)
    for pq in pool_qs:
        if pq not in order:
            order.append(pq)
    order.append(dve_full[-1])
    assert sorted(order) == list(range(NOFF)), order
    return pool_qs, pool_at, dve_full, order

BF16 = ml_dtypes.bfloat16

_PROGRAM = None
LAST_RESULTS = None  # BassKernelResults of the most recent run (for profiling)


def _mult_weight(d: int, p: int, size: int = H) -> int:
    """Number of 3x3 window anchors pairing pixel p with p+d along one axis."""
    lo, hi = max(0, -d), 2 - max(d, 0)
    lo2, hi2 = max(lo, p - (size - KS)), min(hi, p)
    return max(0, hi2 - lo2 + 1)


def _build_program():
    import concourse.tile as tile
    from concourse import bacc, mybir
    from concourse.alu_op_type import AluOpType
    from contextlib import ExitStack

    bf = mybir.dt.bfloat16
    f32 = mybir.dt.float32
    A = AluOpType
    AF = mybir.ActivationFunctionType

    nc = bacc.Bacc("TRN2", target_bir_lowering=False, debug=False,
                   num_devices=NCORES)

    lg_d = nc.dram_tensor("lg", [KP, FDC], bf, kind="ExternalInput")
    # consumption-ordered: WQ block [p*PCOLS,...), then all WM blocks
    wqm_d = nc.dram_tensor("wqm", [128, 2 * NOFF * PCOLS], bf,
                           kind="ExternalInput")
    sel_d = nc.dram_tensor("sel", [KP, NG], bf, kind="ExternalInput")
    out = nc.dram_tensor("out", [128, 2 * NOFF], f32, kind="ExternalOutput")
    out2 = nc.dram_tensor("out2", [128, 16], f32, kind="ExternalOutput")
    dbg = None
    if os.environ.get("AFF_DEBUG_ACC"):
        dbg = nc.dram_tensor("dbg", [128, 2 * NOFF], f32,
                             kind="ExternalOutput")

    pool_qs, pool_at, dve_full, cons_order = _schedule()
    wq_pos = {q: i for i, q in enumerate(cons_order)}
    pool_last = pool_qs[-1] if (pool_qs and POOL_TAKE > 0) else -1

    with ExitStack() as ctx:
        tc = ctx.enter_context(tile.TileContext(nc, pool_alloc_mode="stack"))
        singles = ctx.enter_context(tc.tile_pool(name="singles", bufs=1))
        prodp = ctx.enter_context(tc.tile_pool(
            name="prod", bufs=int(os.environ.get("AFF_PROD_BUFS", "3"))))
        poolprodp = ctx.enter_context(tc.tile_pool(
            name="pprod", bufs=min(3, max(1, len(pool_qs)))))
        scr = ctx.enter_context(tc.tile_pool(
            name="scr", bufs=int(os.environ.get("AFF_SCR_BUFS", "2"))))
        psum = ctx.enter_context(tc.tile_pool(
            name="psum", bufs=int(os.environ.get("AFF_PSUM_BUFS", "4")),
            space="PSUM"))
        psumf = ctx.enter_context(tc.tile_pool(
            name="psumf", bufs=1, space="PSUM"))

        LG = singles.tile([KP, FDC], bf, name="LG")
        ones128 = singles.tile([128, 1], bf, name="ones128")
        fp = psumf.tile([128, 16], f32, name="fp")
        fcp = singles.tile([128, 16], f32, name="fcp")
        WQM = singles.tile([128, 2 * NOFF * PCOLS], bf, name="WQM")
        SEL = singles.tile([KP, NG], bf, name="SEL")
        acc = singles.tile([128, 2 * NOFF], f32, name="acc")

        nc.vector.memset(acc[:], 0.0)
        nc.vector.memset(ones128[:], 1.0)

        # one DMA device: order matters. The LG band gates all compute, so
        # it goes first, laddered in 4 chunks: the first DVE product is
        # emitted in column quarters, each gated only by the chunks it
        # reads. SEL is needed by the first matmul; the first consumed
        # offsets' weights lead, the rest follow.
        S0 = OFFSETS[dve_full[0]][0] * XW + OFFSETS[dve_full[0]][1]
        qtr = PF // 4
        cuts = [0, qtr + S0 + 4, 2 * qtr + S0 + 4, 3 * qtr + S0 + 4, FDC]
        for a, b in zip(cuts[:-1], cuts[1:]):
            nc.sync.dma_start(LG[:, a:b], lg_d[:, a:b])
        nc.sync.dma_start(SEL[:], sel_d[:])
        wq_lead = 2
        nc.sync.dma_start(WQM[:, 0:wq_lead * PCOLS],
                          wqm_d[:, 0:wq_lead * PCOLS])
        nc.sync.dma_start(WQM[:, wq_lead * PCOLS:NOFF * PCOLS],
                          wqm_d[:, wq_lead * PCOLS:NOFF * PCOLS])
        nc.sync.dma_start(WQM[:, NOFF * PCOLS:],
                          wqm_d[:, NOFF * PCOLS:])

        # layout: WQ block cols [p*PCOLS,(p+1)*PCOLS) then WM block
        # NOFF*PCOLS later, p = consumption position (contiguous phases)
        def wq_ap(q):
            p = wq_pos[q]
            return WQM[:, p * PCOLS:(p + 1) * PCOLS]

        def wm_ap(q):
            p = wq_pos[q]
            return WQM[:, (NOFF + p) * PCOLS:(NOFF + p + 1) * PCOLS]

        prods = {}   # q -> product tile (awaiting matmuls)
        psums = {}   # q -> D psum tile (awaiting jd+exp)
        etiles = {}  # q -> exp tile (awaiting ln)
        act_seq = []  # ACT instrs in emission order; chained below so the
        # scheduler can't interleave Ln-table ops into exp-table phases
        # per-offset exp/ln tiles are column slices (consumption order) of
        # two big tiles so phase lns and ws-reductions batch into wide ops
        EB = singles.tile([128, NOFF * PCOLS], bf, name="EB")
        LB = singles.tile([128, NOFF * PCOLS], bf, name="LB")
        DB = singles.tile([128, NOFF * PCOLS], bf, name="DB")

        def esl(q):
            p = wq_pos[q]
            return EB[:, p * PCOLS:(p + 1) * PCOLS]

        def emit_prod(q, engine, pool):
            di, dj, _sym = OFFSETS[q]
            S = di * XW + dj
            prod = pool.tile([KP, PF], bf, tag="prod")
            engine.tensor_tensor(prod[:], LG[:, 0:PF], LG[:, S:S + PF],
                                 A.mult)
            prods[q] = prod

        def emit_mm(q):
            prod = prods.pop(q)
            pt = psum.tile([128, PCOLS], f32, tag="D")
            for k in range(NBLK):
                nc.tensor.matmul(pt[:, NG * k:NG * (k + 1)],
                                 prod[:, 128 * k:128 * (k + 1)],
                                 SEL[:], start=True, stop=True)
            psums[q] = pt

        def emit_jd_exp(q, in_phase_a=False):
            pt = psums.pop(q)
            p = wq_pos[q]
            if q == 0:
                # offset (0,0): D = |v|^2 >= 0 can reach ~80, far past the
                # Ln table's usable range. Labels always self-match (m=1),
                # so w*softplus(D) - w*D = w*ln(1+exp(-D)) exactly: flip the
                # exponent sign (Ln input stays in (1,2]) and the label term
                # cancels - no jd needed (host zeroes its WM block).
                act_seq.append(nc.scalar.activation(esl(q)[:], pt[:],
                                                    AF.Exp, scale=-1.0))
                if in_phase_a:
                    dsl = DB[:, p * PCOLS:(p + 1) * PCOLS]
                    act_seq.append(nc.scalar.activation(dsl[:], pt[:],
                                                        AF.Copy))
            elif in_phase_a:
                # phase-A label terms batch into one wide reduce later; the
                # idle ACT engine parks a bf16 copy of D in DB meanwhile
                act_seq.append(nc.scalar.activation(esl(q)[:], pt[:],
                                                    AF.Exp))
                dsl = DB[:, p * PCOLS:(p + 1) * PCOLS]
                act_seq.append(nc.scalar.activation(dsl[:], pt[:], AF.Copy))
            else:
                # tail offsets: per-offset label term straight from PSUM
                jscr = scr.tile([128, PCOLS], bf, tag="jscr")
                nc.vector.scalar_tensor_tensor(
                    jscr[:], pt[:], 1.0, wm_ap(q), A.bypass, A.mult,
                    accum_out=acc[:, NOFF + p:NOFF + p + 1])
                act_seq.append(nc.scalar.activation(esl(q)[:], pt[:],
                                                    AF.Exp))
            etiles[q] = True

        def _phase_cols(qs):
            ps = sorted(wq_pos[q] for q in qs)
            assert ps == list(range(ps[0], ps[0] + len(ps))), ps
            return ps[0] * PCOLS, (ps[0] + len(ps)) * PCOLS

        def emit_ln_phase(qs):
            # one wide Ln over the phase's contiguous exp columns
            a, b = _phase_cols(qs)
            act_seq.append(nc.scalar.activation(LB[:, a:b], EB[:, a:b],
                                                AF.Ln, bias=1.0))
            for q in qs:
                etiles.pop(q, None)

        def _pe_colsum(src_ap, a, b, fcol, first=True):
            # per-partition column sum via TensorE ones-contraction,
            # PSUM-accumulated over 128-column chunks; first=False keeps
            # accumulating onto the column's existing group results
            n = b - a
            chunks = [(a + j * 128, min(a + (j + 1) * 128, b))
                      for j in range((n + 127) // 128)]
            for ci, (c0, c1) in enumerate(chunks):
                nc.tensor.matmul(fp[0:c1 - c0, fcol:fcol + 1],
                                 src_ap[:, c0:c1], ones128[:],
                                 start=(first and ci == 0),
                                 stop=(ci == len(chunks) - 1))

        def emit_jd_phase(qs):
            # phase-A label terms: 2x-mode multiply of the D copies by the
            # contiguous WM blocks (offset 0's WM zeroed host-side), with
            # the column sum on the idle TensorE
            a, b = _phase_cols(qs)
            jscr = scr.tile([128, NOFF * PCOLS], bf, tag="jscr2")
            nc.vector.tensor_tensor(
                jscr[:, a:b], DB[:, a:b],
                WQM[:, NOFF * PCOLS + a:NOFF * PCOLS + b], A.mult)
            _pe_colsum(jscr, a, b, 0)

        def emit_ws_phase(qs, wide=False):
            a, b = _phase_cols(qs)
            col = a // PCOLS
            wscr = scr.tile([128, NOFF * PCOLS], bf, tag="wscr")
            if wide:
                nc.vector.tensor_tensor(
                    wscr[:, a:b], LB[:, a:b], WQM[:, a:b], A.mult)
                _pe_colsum(wscr, a, b, 8)
            else:
                # tail offsets: 2x multiply, then accumulate onto the
                # phase-A softplus column's standing PSUM group
                nc.vector.tensor_tensor(
                    wscr[:, a:b], LB[:, a:b], WQM[:, a:b], A.mult)
                _pe_colsum(wscr, a, b, 8, first=False)

        # ---- emission schedule ----------------------------------------
        # pool products start immediately and run concurrently with the
        # DVE; the last one leaves its trailing POOL_TAKE columns to a DVE
        # remainder op emitted at loop index REM_AT
        for q in pool_qs:
            di, dj, _sym = OFFSETS[q]
            S = di * XW + dj
            hi = PF - POOL_TAKE if q == pool_last else PF
            prod = poolprodp.tile([KP, PF], bf, tag="prod")
            nc.gpsimd.tensor_tensor(prod[:, 0:hi], LG[:, 0:hi],
                                    LG[:, S:S + hi], A.mult)
            prods[q] = prod
        lastq = dve_full[-1]
        last_prod = None
        if POOL_PRE > 0:
            # after its queue drains, the pool precomputes the head of the
            # last DVE offset's product; the DVE fills in the tail
            di, dj, _sym = OFFSETS[lastq]
            S = di * XW + dj
            last_prod = prodp.tile([KP, PF], bf, tag="prod")
            nc.gpsimd.tensor_tensor(last_prod[:, 0:POOL_PRE],
                                    LG[:, 0:POOL_PRE],
                                    LG[:, S:S + POOL_PRE], A.mult)

        exp_order = []   # offsets in ACT-exp emission order
        phase_a = None   # offsets ln'd in the batched first phase
        jd_a_done = False

        for i, q in enumerate(dve_full):
            if i == 0:
                # first product emitted in column quarters so each runs as
                # soon as the LG DMA chunk covering its reads lands
                di, dj, _sym = OFFSETS[q]
                S = di * XW + dj
                prod = prodp.tile([KP, PF], bf, tag="prod")
                for a in range(0, PF, qtr):
                    nc.vector.tensor_tensor(prod[:, a:a + qtr],
                                            LG[:, a:a + qtr],
                                            LG[:, S + a:S + a + qtr], A.mult)
                prods[q] = prod
            elif q == lastq and last_prod is not None:
                di, dj, _sym = OFFSETS[q]
                S = di * XW + dj
                nc.vector.tensor_tensor(
                    last_prod[:, POOL_PRE:PF], LG[:, POOL_PRE:PF],
                    LG[:, S + POOL_PRE:S + PF], A.mult)
                prods[q] = last_prod
            else:
                emit_prod(q, nc.vector, prodp)
            if i == REM_AT and pool_last >= 0:
                di, dj, _sym = OFFSETS[pool_last]
                S = di * XW + dj
                pl = prods[pool_last]
                nc.vector.tensor_tensor(
                    pl[:, PF - POOL_TAKE:PF], LG[:, PF - POOL_TAKE:PF],
                    LG[:, S + PF - POOL_TAKE:S + PF], A.mult)
            if i >= 1:
                prev = dve_full[i - 1]
                emit_jd_exp(prev, in_phase_a=(phase_a is None))
                exp_order.append(prev)
            for pq in pool_qs:
                if pool_at.get(pq) == i:
                    emit_mm(pq)
                    emit_jd_exp(pq, in_phase_a=(phase_a is None))
                    exp_order.append(pq)
            emit_mm(q)
            if i == LN_SPLIT:
                phase_a = list(exp_order)
                emit_ln_phase(phase_a)
            if i == LN_SPLIT + 1 and phase_a is not None:
                emit_jd_phase(phase_a)
                jd_a_done = True

        # tail: leftover pool offsets, the last DVE offset, then phase B
        # per-offset (short serial chain) while the batched phase-A
        # weighted sum fills the DVE
        for pq in pool_qs:
            if pq in prods:
                emit_mm(pq)
                emit_jd_exp(pq)
                exp_order.append(pq)
        emit_jd_exp(dve_full[-1])
        exp_order.append(dve_full[-1])
        if phase_a is not None and not jd_a_done:
            emit_jd_phase(phase_a)
        phase_b = [qq for qq in exp_order if qq in etiles]
        if os.environ.get("AFF_B_BATCH", "0") == "1":
            if phase_b:
                emit_ln_phase(phase_b)
            if phase_a is not None:
                emit_ws_phase(phase_a, wide=True)
            if phase_b:
                emit_ws_phase(phase_b)
        else:
            for qq in phase_b:
                emit_ln_phase([qq])
            if phase_a is not None:
                emit_ws_phase(phase_a, wide=True)
            for qq in phase_b:
                emit_ws_phase([qq])

        from concourse.tile import add_dep_helper
        for i in range(1, len(act_seq)):
            add_dep_helper(act_seq[i].ins, act_seq[i - 1].ins, sync=False,
                           reason="ACT emission order (table-set phases)")

        if dbg is not None:
            nc.sync.dma_start(dbg[:], acc[:])
        # the [128, 26] accumulator goes to the host, which does the final
        # signed sum (cheaper than a matmul+reduce tail on-device)
        nc.vector.tensor_copy(fcp[:], fp[:])
        nc.sync.dma_start(out[:], acc[:])
        nc.scalar.dma_start(out2[:], fcp[:])
    nc.compile()
    return nc


def _host_inputs(logits: np.ndarray, labels: np.ndarray):
    logits = np.asarray(logits, dtype=np.float32)
    labels = np.asarray(labels)
    lg_bf = logits.astype(BF16)

    wy_tab = np.array([[_mult_weight(d, p, H) for p in range(H)]
                       for d in range(3)], dtype=np.float32)
    wx_tab = np.array([[_mult_weight(d, p, W) for p in range(W)]
                       for d in range(-2, 3)], dtype=np.float32)

    sel = np.zeros((KP, NG), dtype=BF16)
    for g in range(NG):
        sel[g * C:(g + 1) * C, g] = 1.0

    # pixel coordinate tables for the block layout
    f_idx = np.arange(PF)
    y_f = f_idx // XW              # band-local y
    xp_f = f_idx % XW              # padded x'
    part_f = f_idx % 128
    blk_f = f_idx // 128
    x_f = xp_f - 2
    valid_f = (y_f < BY) & (x_f >= 0) & (x_f < W)

    in_maps = []
    for core in range(NCORES):
        r0 = core * BPC
        lg = np.zeros((KP, FDC), dtype=BF16)
        band = np.zeros((NB, N, C, TY, XW), dtype=BF16)
        for b in range(NB):
            rows = max(0, min(TY, H - (r0 + BY * b)))
            band[b, :, :, :rows, 2:2 + W] = \
                lg_bf[:, :, r0 + BY * b:r0 + BY * b + rows, :]
        lg[:, 0:TY * XW] = band.reshape(KP, TY * XW)
        in_map = {"lg": lg, "sel": sel}

        cons_order = _schedule()[-1]
        wq_pos = {q: i for i, q in enumerate(cons_order)}
        wqm = np.zeros((128, 2 * NOFF * PCOLS), dtype=np.float32)
        for q, (di, dj, sym) in enumerate(OFFSETS):
            for g in range(NG):
                b, n = g // N, g % N
                Y = r0 + BY * b + y_f
                X = x_f
                Yc = np.clip(Y, 0, H - 1)
                Xc = np.clip(X, 0, W - 1)
                Y2 = np.clip(Y + di, 0, H - 1)
                X2 = np.clip(X + dj, 0, W - 1)
                w = sym * wy_tab[di, Yc] * wx_tab[dj + 2, Xc] * valid_f
                m = (labels[n, Yc, Xc] == labels[n, Y2, X2])
                cols = wq_pos[q] * PCOLS + NG * blk_f + g
                wqm[part_f, cols] = w
                # offset (0,0)'s label term cancels into its softplus form;
                # its WM block must be zero for the batched phase-A reduce
                wqm[part_f, cols + NOFF * PCOLS] = 0.0 if q == 0 else w * m
        in_map["wqm"] = wqm.astype(BF16)
        in_maps.append(in_map)
    return in_maps


def kernel(logits: np.ndarray, labels: np.ndarray) -> np.ndarray:
    global _PROGRAM, LAST_RESULTS
    from concourse.bass_utils import run_bass_kernel_spmd

    if _PROGRAM is None:
        _PROGRAM = _build_program()

    in_maps = _host_inputs(logits, labels)
    trace = bool(int(os.environ.get("AFF_TRACE", "0")))
    results = run_bass_kernel_spmd(
        _PROGRAM, in_maps, core_ids=list(range(NCORES)), trace=trace)
    LAST_RESULTS = results

    total = 0.0
    for r in results.results:
        a = np.asarray(r["out"], dtype=np.float64)
        b = np.asarray(r["out2"], dtype=np.float64)
        total += float(a[:, 0:NOFF].sum() - a[:, NOFF:2 * NOFF].sum())
        total += float(b[:, 8].sum() - b[:, 0].sum())
    Lwin = (H - KS + 1) * (W - KS + 1)
    return np.float32(total / (N * KS**4 * Lwin))


# revision 56
# speedup vs baseline: 1.0113x; 1.0013x over previous
"""Trainium2 Bass kernel for AffinityLoss (nn_AffinityLoss_70875550318911).

Math: loss = mean over (n, a, b, l) of BCEWithLogits(aff_map, lb_map) where
aff_map[n,a,b,l] = sum_c lu[n,c,a,l]*lu[n,c,b,l] over 3x3 unfold positions.

Reformulation: pairs (a,b) sharing relative offset d=(di,dj) share one
correlation map D_d[p] = sum_c logits[c,p]*logits[c,p+d]; by symmetry only 13
offsets are needed. Border multiplicities w(p) = sym*wy(di,Y)*wx(dj,X), and
with m = [labels match at offset d] the BCE term reduces to

  total = sum_d sum_p [ w*softplus(D) - (w*m)*D ],   softplus(x)=ln(1+e^x)
  loss  = total / (n * 81 * 382^2)

Layout: channels-on-partitions. Per core, 6 groups (3 y-bands x 2 batches)
of 19 channels -> 114 partitions; free dim = flattened (y, x) band pixels,
so ALL 13 offset shifts are free-dim shifts of ONE resident tile (one input
DMA, no shifted copies). Engine split:
  - DVE (+ Pool for a few offsets) computes the elementwise product tile.
  - TensorE contracts the 19 channels: per 128-column block, a matmul with
    stationary = product block [114,128] and moving = a constant 6-column
    group-selector produces D for 128 pixels x 6 groups straight into PSUM
    (f32, exact sum of bf16 products) - replacing the DVE c-sum add tree.
  - ACT computes softplus as Exp then Ln(bias=1), batched in two phases so
    only 4 activation-table loads occur (exp and ln live in different
    table sets; the merged natural_log_exp set's Ln is broken on HW).
  - DVE scalar_tensor_tensor ops with accum_out form the two weighted sums
    per offset (weights fully host-precomputed per pixel, including all
    border effects - no correction terms needed).
A final ones-matmul reduces the per-partition accumulator strips; each core
returns one unnormalized scalar; host sums and scales.

Emission order is software-pipelined: consumers of offset q's PSUM are
emitted roughly when that PSUM becomes ready, so no engine FIFO head-blocks.
"""
import os
import numpy as np
import ml_dtypes

NCORES = 8
N, C, H, W = 2, 19, 384, 384
KS = 3
BPC = H // NCORES           # 48 owned rows per core
NB = 3                      # y-bands per core
BY = BPC // NB              # 16 owned rows per band
TY = BY + 2                 # 18 band rows incl halo
XW = W + 4                  # padded row width; x' = x + 2
NG = NB * N                 # 6 groups (band, batch)
KP = NG * C                 # 114 partitions
NBLK = 49                   # 128-wide pixel blocks per offset
PF = NBLK * 128             # 6272 product free size
FDC = 2 * XW + 2 + PF + 2   # 7052: max shift 778, +pad (even)
PCOLS = NG * NBLK           # 294 psum columns
NOFF = 13

# (di, dj, sym): di >= 0; for di == 0 only dj >= 0. sym 2 covers (-di,-dj).
OFFSETS = [(0, 0, 1.0), (0, 1, 2.0), (0, 2, 2.0),
           (1, -2, 2.0), (1, -1, 2.0), (1, 0, 2.0), (1, 1, 2.0), (1, 2, 2.0),
           (2, -2, 2.0), (2, -1, 2.0), (2, 0, 2.0), (2, 1, 2.0), (2, 2, 2.0)]

# offsets whose product runs on the GPSIMD (Pool) engine, freeing the DVE
POOL_QS = tuple(int(x) for x in
                os.environ.get("AFF_POOL_QS", "0,1,2").split(",") if x != "")
# after which DVE-product index each pool offset's consumers are emitted
POOL_AT = tuple(int(x) for x in
                os.environ.get("AFF_POOL_AT", "4,7,9").split(",") if x != "")
# trailing columns of the LAST pool offset's product computed by the DVE
# instead (0 disables), and the DVE-product index where that runs
POOL_TAKE = int(os.environ.get("AFF_POOL_TAKE", "2304"))
REM_AT = int(os.environ.get("AFF_REM_AT", "7"))
# leading columns of the LAST DVE offset's product precomputed by the Pool
# after its own products finish (0 disables)
POOL_PRE = int(os.environ.get("AFF_POOL_PRE", "1152"))
# DVE-product index after which the batched phase-A Ln is emitted
LN_SPLIT = int(os.environ.get("AFF_LN_SPLIT", "7"))


def _schedule():
    # -> (pool_qs, pool_at, dve_full, cons_order): emission schedule and
    # the offset consumption order (= wqm/EB/LB column layout order)
    pool_qs = [q for q in POOL_QS if 0 <= q < NOFF]
    dve_full = [q for q in range(NOFF) if q not in pool_qs]
    pool_at = {q: POOL_AT[i] for i, q in enumerate(pool_qs)
               if i < len(POOL_AT)}
    order = []
    for i in range(len(dve_full)):
        if i >= 1:
            order.append(dve_full[i - 1])
        for pq in pool_qs:
            if pool_at.get(pq) == i:
                order.append(pq)
    for pq in pool_qs:
        if pq not in order:
            order.append(pq)
    order.append(dve_full[-1])
    assert sorted(order) == list(range(NOFF)), order
    return pool_qs, pool_at, dve_full, order

BF16 = ml_dtypes.bfloat16

_PROGRAM = None
LAST_RESULTS = None  # BassKernelResults of the most recent run (for profiling)


def _mult_weight(d: int, p: int, size: int = H) -> int:
    """Number of 3x3 window anchors pairing pixel p with p+d along one axis."""
    lo, hi = max(0, -d), 2 - max(d, 0)
    lo2, hi2 = max(lo, p - (size - KS)), min(hi, p)
    return max(0, hi2 - lo2 + 1)


def _build_program():
    import concourse.tile as tile
    from concourse import bacc, mybir
    from concourse.alu_op_type import AluOpType
    from contextlib import ExitStack

    bf = mybir.dt.bfloat16
    f32 = mybir.dt.float32
    A = AluOpType
    AF = mybir.ActivationFunctionType

    nc = bacc.Bacc("TRN2", target_bir_lowering=False, debug=False,
                   num_devices=NCORES)

    lg_d = nc.dram_tensor("lg", [KP, FDC], bf, kind="ExternalInput")
    # consumption-ordered: WQ block [p*PCOLS,...), then all WM blocks
    wqm_d = nc.dram_tensor("wqm", [128, 2 * NOFF * PCOLS], bf,
                           kind="ExternalInput")
    sel_d = nc.dram_tensor("sel", [KP, NG], bf, kind="ExternalInput")
    out = nc.dram_tensor("out", [128, 2 * NOFF], f32, kind="ExternalOutput")
    out2 = nc.dram_tensor("out2", [128, 16], f32, kind="ExternalOutput")
    dbg = None
    if os.environ.get("AFF_DEBUG_ACC"):
        dbg = nc.dram_tensor("dbg", [128, 2 * NOFF], f32,
                             kind="ExternalOutput")

    pool_qs, pool_at, dve_full, cons_order = _schedule()
    wq_pos = {q: i for i, q in enumerate(cons_order)}
    pool_last = pool_qs[-1] if (pool_qs and POOL_TAKE > 0) else -1

    with ExitStack() as ctx:
        tc = ctx.enter_context(tile.TileContext(nc, pool_alloc_mode="stack"))
        singles = ctx.enter_context(tc.tile_pool(name="singles", bufs=1))
        prodp = ctx.enter_context(tc.tile_pool(
            name="prod", bufs=int(os.environ.get("AFF_PROD_BUFS", "3"))))
        poolprodp = ctx.enter_context(tc.tile_pool(
            name="pprod", bufs=min(3, max(1, len(pool_qs)))))
        scr = ctx.enter_context(tc.tile_pool(
            name="scr", bufs=int(os.environ.get("AFF_SCR_BUFS", "2"))))
        psum = ctx.enter_context(tc.tile_pool(
            name="psum", bufs=int(os.environ.get("AFF_PSUM_BUFS", "4")),
            space="PSUM"))
        psumf = ctx.enter_context(tc.tile_pool(
            name="psumf", bufs=1, space="PSUM"))

        LG = singles.tile([KP, FDC], bf, name="LG")
        ones128 = singles.tile([128, 1], bf, name="ones128")
        fp = psumf.tile([128, 16], f32, name="fp")
        fcp = singles.tile([128, 16], f32, name="fcp")
        WQM = singles.tile([128, 2 * NOFF * PCOLS], bf, name="WQM")
        SEL = singles.tile([KP, NG], bf, name="SEL")
        acc = singles.tile([128, 2 * NOFF], f32, name="acc")

        nc.vector.memset(acc[:], 0.0)
        nc.vector.memset(ones128[:], 1.0)

        # one DMA device: order matters. The LG band gates all compute, so
        # it goes first, laddered in 4 chunks: the first DVE product is
        # emitted in column quarters, each gated only by the chunks it
        # reads. SEL is needed by the first matmul; the first consumed
        # offsets' weights lead, the rest follow.
        S0 = OFFSETS[dve_full[0]][0] * XW + OFFSETS[dve_full[0]][1]
        qtr = PF // 4
        cuts = [0, qtr + S0 + 4, 2 * qtr + S0 + 4, 3 * qtr + S0 + 4, FDC]
        for a, b in zip(cuts[:-1], cuts[1:]):
            nc.sync.dma_start(LG[:, a:b], lg_d[:, a:b])
        nc.sync.dma_start(SEL[:], sel_d[:])
        wq_lead = 2
        nc.sync.dma_start(WQM[:, 0:wq_lead * PCOLS],
                          wqm_d[:, 0:wq_lead * PCOLS])
        nc.sync.dma_start(WQM[:, wq_lead * PCOLS:NOFF * PCOLS],
                          wqm_d[:, wq_lead * PCOLS:NOFF * PCOLS])
        nc.sync.dma_start(WQM[:, NOFF * PCOLS:],
                          wqm_d[:, NOFF * PCOLS:])

        # layout: WQ block cols [p*PCOLS,(p+1)*PCOLS) then WM block
        # NOFF*PCOLS later, p = consumption position (contiguous phases)
        def wq_ap(q):
            p = wq_pos[q]
            return WQM[:, p * PCOLS:(p + 1) * PCOLS]

        def wm_ap(q):
            p = wq_pos[q]
            return WQM[:, (NOFF + p) * PCOLS:(NOFF + p + 1) * PCOLS]

        prods = {}   # q -> product tile (awaiting matmuls)
        psums = {}   # q -> D psum tile (awaiting jd+exp)
        etiles = {}  # q -> exp tile (awaiting ln)
        act_seq = []  # ACT instrs in emission order; chained below so the
        # scheduler can't interleave Ln-table ops into exp-table phases
        # per-offset exp/ln tiles are column slices (consumption order) of
        # two big tiles so phase lns and ws-reductions batch into wide ops
        EB = singles.tile([128, NOFF * PCOLS], bf, name="EB")
        LB = singles.tile([128, NOFF * PCOLS], bf, name="LB")
        DB = singles.tile([128, NOFF * PCOLS], bf, name="DB")

        def esl(q):
            p = wq_pos[q]
            return EB[:, p * PCOLS:(p + 1) * PCOLS]

        def emit_prod(q, engine, pool):
            di, dj, _sym = OFFSETS[q]
            S = di * XW + dj
            prod = pool.tile([KP, PF], bf, tag="prod")
            engine.tensor_tensor(prod[:], LG[:, 0:PF], LG[:, S:S + PF],
                                 A.mult)
            prods[q] = prod

        def emit_mm(q):
            prod = prods.pop(q)
            pt = psum.tile([128, PCOLS], f32, tag="D")
            for k in range(NBLK):
                nc.tensor.matmul(pt[:, NG * k:NG * (k + 1)],
                                 prod[:, 128 * k:128 * (k + 1)],
                                 SEL[:], start=True, stop=True)
            psums[q] = pt

        def emit_jd_exp(q, in_phase_a=False):
            pt = psums.pop(q)
            p = wq_pos[q]
            if q == 0:
                # offset (0,0): D = |v|^2 >= 0 can reach ~80, far past the
                # Ln table's usable range. Labels always self-match (m=1),
                # so w*softplus(D) - w*D = w*ln(1+exp(-D)) exactly: flip the
                # exponent sign (Ln input stays in (1,2]) and the label term
                # cancels - no jd needed (host zeroes its WM block).
                act_seq.append(nc.scalar.activation(esl(q)[:], pt[:],
                                                    AF.Exp, scale=-1.0))
                if in_phase_a:
                    dsl = DB[:, p * PCOLS:(p + 1) * PCOLS]
                    act_seq.append(nc.scalar.activation(dsl[:], pt[:],
                                                        AF.Copy))
            elif in_phase_a:
                # phase-A label terms batch into one wide reduce later; the
                # idle ACT engine parks a bf16 copy of D in DB meanwhile
                act_seq.append(nc.scalar.activation(esl(q)[:], pt[:],
                                                    AF.Exp))
                dsl = DB[:, p * PCOLS:(p + 1) * PCOLS]
                act_seq.append(nc.scalar.activation(dsl[:], pt[:], AF.Copy))
            else:
                # tail offsets: per-offset label term straight from PSUM
                jscr = scr.tile([128, PCOLS], bf, tag="jscr")
                nc.vector.scalar_tensor_tensor(
                    jscr[:], pt[:], 1.0, wm_ap(q), A.bypass, A.mult,
                    accum_out=acc[:, NOFF + p:NOFF + p + 1])
                act_seq.append(nc.scalar.activation(esl(q)[:], pt[:],
                                                    AF.Exp))
            etiles[q] = True

        def _phase_cols(qs):
            ps = sorted(wq_pos[q] for q in qs)
            assert ps == list(range(ps[0], ps[0] + len(ps))), ps
            return ps[0] * PCOLS, (ps[0] + len(ps)) * PCOLS

        def emit_ln_phase(qs):
            # one wide Ln over the phase's contiguous exp columns
            a, b = _phase_cols(qs)
            act_seq.append(nc.scalar.activation(LB[:, a:b], EB[:, a:b],
                                                AF.Ln, bias=1.0))
            for q in qs:
                etiles.pop(q, None)

        def _pe_colsum(src_ap, a, b, fcol, first=True):
            # per-partition column sum via TensorE ones-contraction,
            # PSUM-accumulated over 128-column chunks; first=False keeps
            # accumulating onto the column's existing group results
            n = b - a
            chunks = [(a + j * 128, min(a + (j + 1) * 128, b))
                      for j in range((n + 127) // 128)]
            for ci, (c0, c1) in enumerate(chunks):
                nc.tensor.matmul(fp[0:c1 - c0, fcol:fcol + 1],
                                 src_ap[:, c0:c1], ones128[:],
                                 start=(first and ci == 0),
                                 stop=(ci == len(chunks) - 1))

        def emit_jd_phase(qs):
            # phase-A label terms: 2x-mode multiply of the D copies by the
            # contiguous WM blocks (offset 0's WM zeroed host-side), with
            # the column sum on the idle TensorE
            a, b = _phase_cols(qs)
            jscr = scr.tile([128, NOFF * PCOLS], bf, tag="jscr2")
            nc.vector.tensor_tensor(
                jscr[:, a:b], DB[:, a:b],
                WQM[:, NOFF * PCOLS + a:NOFF * PCOLS + b], A.mult)
            _pe_colsum(jscr, a, b, 0)

        def emit_ws_phase(qs, wide=False):
            a, b = _phase_cols(qs)
            col = a // PCOLS
            wscr = scr.tile([128, NOFF * PCOLS], bf, tag="wscr")
            if wide:
                nc.vector.tensor_tensor(
                    wscr[:, a:b], LB[:, a:b], WQM[:, a:b], A.mult)
                _pe_colsum(wscr, a, b, 8)
            else:
                # tail offsets: 2x multiply, then accumulate onto the
                # phase-A softplus column's standing PSUM group
                nc.vector.tensor_tensor(
                    wscr[:, a:b], LB[:, a:b], WQM[:, a:b], A.mult)
                _pe_colsum(wscr, a, b, 8, first=False)

        # ---- emission schedule ----------------------------------------
        # pool products start immediately and run concurrently with the
        # DVE; the last one leaves its trailing POOL_TAKE columns to a DVE
        # remainder op emitted at loop index REM_AT
        for q in pool_qs:
            di, dj, _sym = OFFSETS[q]
            S = di * XW + dj
            hi = PF - POOL_TAKE if q == pool_last else PF
            prod = poolprodp.tile([KP, PF], bf, tag="prod")
            nc.gpsimd.tensor_tensor(prod[:, 0:hi], LG[:, 0:hi],
                                    LG[:, S:S + hi], A.mult)
            prods[q] = prod
        lastq = dve_full[-1]
        last_prod = None
        if POOL_PRE > 0:
            # after its queue drains, the pool precomputes the head of the
            # last DVE offset's product; the DVE fills in the tail
            di, dj, _sym = OFFSETS[lastq]
            S = di * XW + dj
            last_prod = prodp.tile([KP, PF], bf, tag="prod")
            nc.gpsimd.tensor_tensor(last_prod[:, 0:POOL_PRE],
                                    LG[:, 0:POOL_PRE],
                                    LG[:, S:S + POOL_PRE], A.mult)

        exp_order = []   # offsets in ACT-exp emission order
        phase_a = None   # offsets ln'd in the batched first phase
        jd_a_done = False

        for i, q in enumerate(dve_full):
            if i == 0:
                # first product emitted in column quarters so each runs as
                # soon as the LG DMA chunk covering its reads lands
                di, dj, _sym = OFFSETS[q]
                S = di * XW + dj
                prod = prodp.tile([KP, PF], bf, tag="prod")
                for a in range(0, PF, qtr):
                    nc.vector.tensor_tensor(prod[:, a:a + qtr],
                                            LG[:, a:a + qtr],
                                            LG[:, S + a:S + a + qtr], A.mult)
                prods[q] = prod
            elif q == lastq and last_prod is not None:
                di, dj, _sym = OFFSETS[q]
                S = di * XW + dj
                nc.vector.tensor_tensor(
                    last_prod[:, POOL_PRE:PF], LG[:, POOL_PRE:PF],
                    LG[:, S + POOL_PRE:S + PF], A.mult)
                prods[q] = last_prod
            else:
                emit_prod(q, nc.vector, prodp)
            if i == REM_AT and pool_last >= 0:
                di, dj, _sym = OFFSETS[pool_last]
                S = di * XW + dj
                pl = prods[pool_last]
                nc.vector.tensor_tensor(
                    pl[:, PF - POOL_TAKE:PF], LG[:, PF - POOL_TAKE:PF],
                    LG[:, S + PF - POOL_TAKE:S + PF], A.mult)
            if i >= 1:
                prev = dve_full[i - 1]
                emit_jd_exp(prev, in_phase_a=(phase_a is None))
                exp_order.append(prev)
            for pq in pool_qs:
                if pool_at.get(pq) == i:
                    emit_mm(pq)
                    emit_jd_exp(pq, in_phase_a=(phase_a is None))
                    exp_order.append(pq)
            emit_mm(q)
            if i == LN_SPLIT:
                phase_a = list(exp_order)
                emit_ln_phase(phase_a)
            if i == LN_SPLIT + 1 and phase_a is not None:
                emit_jd_phase(phase_a)
                jd_a_done = True

        # tail: leftover pool offsets, the last DVE offset, then phase B
        # per-offset (short serial chain) while the batched phase-A
        # weighted sum fills the DVE
        for pq in pool_qs:
            if pq in prods:
                emit_mm(pq)
                emit_jd_exp(pq)
                exp_order.append(pq)
        emit_jd_exp(dve_full[-1])
        exp_order.append(dve_full[-1])
        if phase_a is not None and not jd_a_done:
            emit_jd_phase(phase_a)
        phase_b = [qq for qq in exp_order if qq in etiles]
        if os.environ.get("AFF_B_BATCH", "0") == "1":
            if phase_b:
                emit_ln_phase(phase_b)
            if phase_a is not None:
                emit_ws_phase(phase_a, wide=True)
            if phase_b:
                emit_ws_phase(phase_b)
        else:
            for qq in phase_b:
                emit_ln_phase([qq])
            if phase_a is not None:
                emit_ws_phase(phase_a, wide=True)
            for qq in phase_b:
                emit_ws_phase([qq])

        from concourse.tile import add_dep_helper
        for i in range(1, len(act_seq)):
            add_dep_helper(act_seq[i].ins, act_seq[i - 1].ins, sync=False,
                           reason="ACT emission order (table-set phases)")

        if dbg is not None:
            nc.sync.dma_start(dbg[:], acc[:])
        # the [128, 26] accumulator goes to the host, which does the final
        # signed sum (cheaper than a matmul+reduce tail on-device)
        nc.vector.tensor_copy(fcp[:], fp[:])
        nc.sync.dma_start(out[:], acc[:])
        nc.scalar.dma_start(out2[:], fcp[:])
    nc.compile()
    return nc


def _host_inputs(logits: np.ndarray, labels: np.ndarray):
    logits = np.asarray(logits, dtype=np.float32)
    labels = np.asarray(labels)
    lg_bf = logits.astype(BF16)

    wy_tab = np.array([[_mult_weight(d, p, H) for p in range(H)]
                       for d in range(3)], dtype=np.float32)
    wx_tab = np.array([[_mult_weight(d, p, W) for p in range(W)]
                       for d in range(-2, 3)], dtype=np.float32)

    sel = np.zeros((KP, NG), dtype=BF16)
    for g in range(NG):
        sel[g * C:(g + 1) * C, g] = 1.0

    # pixel coordinate tables for the block layout
    f_idx = np.arange(PF)
    y_f = f_idx // XW              # band-local y
    xp_f = f_idx % XW              # padded x'
    part_f = f_idx % 128
    blk_f = f_idx // 128
    x_f = xp_f - 2
    valid_f = (y_f < BY) & (x_f >= 0) & (x_f < W)

    in_maps = []
    for core in range(NCORES):
        r0 = core * BPC
        lg = np.zeros((KP, FDC), dtype=BF16)
        band = np.zeros((NB, N, C, TY, XW), dtype=BF16)
        for b in range(NB):
            rows = max(0, min(TY, H - (r0 + BY * b)))
            band[b, :, :, :rows, 2:2 + W] = \
                lg_bf[:, :, r0 + BY * b:r0 + BY * b + rows, :]
        lg[:, 0:TY * XW] = band.reshape(KP, TY * XW)
        in_map = {"lg": lg, "sel": sel}

        cons_order = _schedule()[-1]
        wq_pos = {q: i for i, q in enumerate(cons_order)}
        wqm = np.zeros((128, 2 * NOFF * PCOLS), dtype=np.float32)
        for q, (di, dj, sym) in enumerate(OFFSETS):
            for g in range(NG):
                b, n = g // N, g % N
                Y = r0 + BY * b + y_f
                X = x_f
                Yc = np.clip(Y, 0, H - 1)
                Xc = np.clip(X, 0, W - 1)
                Y2 = np.clip(Y + di, 0, H - 1)
                X2 = np.clip(X + dj, 0, W - 1)
                w = sym * wy_tab[di, Yc] * wx_tab[dj + 2, Xc] * valid_f
                m = (labels[n, Yc, Xc] == labels[n, Y2, X2])
                cols = wq_pos[q] * PCOLS + NG * blk_f + g
                wqm[part_f, cols] = w
                # offset (0,0)'s label term cancels into its softplus form;
                # its WM block must be zero for the batched phase-A reduce
                wqm[part_f, cols + NOFF * PCOLS] = 0.0 if q == 0 else w * m
        in_map["wqm"] = wqm.astype(BF16)
        in_maps.append(in_map)
    return in_maps


def kernel(logits: np.ndarray, labels: np.ndarray) -> np.ndarray:
    global _PROGRAM, LAST_RESULTS
    from concourse.bass_utils import run_bass_kernel_spmd

    if _PROGRAM is None:
        _PROGRAM = _build_program()

    in_maps = _host_inputs(logits, labels)
    trace = bool(int(os.environ.get("AFF_TRACE", "0")))
    results = run_bass_kernel_spmd(
        _PROGRAM, in_maps, core_ids=list(range(NCORES)), trace=trace)
    LAST_RESULTS = results

    total = 0.0
    for r in results.results:
        a = np.asarray(r["out"], dtype=np.float64)
        b = np.asarray(r["out2"], dtype=np.float64)
        total += float(a[:, 0:NOFF].sum() - a[:, NOFF:2 * NOFF].sum())
        total += float(b[:, 8].sum() - b[:, 0].sum())
    Lwin = (H - KS + 1) * (W - KS + 1)
    return np.float32(total / (N * KS**4 * Lwin))
